# revision 15
# baseline (speedup 1.0000x reference)
"""Trainium2 Bass kernel for ConvexDisplacementUpdate (B=4, L=4096, D=256).

new_coords = alpha * softmax(10 * qhat @ khat^T) @ coords + (1-alpha) * coords
q = l2norm(latents @ Wq^T), k = l2norm(latents @ Wk^T)  (row-wise l2norm)

Strategy (flash-attention style; the [L, L] score matrix never touches HBM):
  - 8 cores = (4 batches) x (2 query halves of 2048 rows). Host rolls each
    core's per-batch data so its own query rows are always columns 0:2048
    of the transposed latents -> one SPMD program, no per-core control flow.
  - Scores are computed transposed, S^T[m, l] = k_m . qhat_l, with k left
    UN-normalized; the per-m factor 10/||k_m|| is a per-partition scale
    folded into the exp() activation.
  - q/k are stored fp8e4m3 (scaled by 16 resp. 4 to center the fp8 range)
    in DoubleRow layout [128, 2, len]: both K=128 contraction halves are
    contracted in ONE PE pass at 2 MACs/cell/cycle -> the score matmuls run
    ~2x faster than the bf16 version (measured ~32us for all scores).
  - softmax without max-subtraction (scores stay in [-4, 4] for this data,
    exp() is safe in fp32 and P fits bf16 easily).
  - numerator and denominator come from one PE matmul per (tile, l-block)
    with the merged hi/lo coords [x_hi, y_hi, 1, x_lo, y_lo] (bf16) as the
    stationary operand, accumulated over all 32 m-tiles in PSUM.
  - final alpha-blend + division happen on host (B*L*2 elements, trivial).

Measured (interleaved A/B loop-differencing, median of 9 rounds, same
session as a 158.6us baseline measurement): ~92.4us.
"""

import numpy as np

B, L, D = 4, 4096, 256
HALF = L // 2  # 2048 query rows per core
NCORES = 8
INV_TEMP = 10.0
QS = 16.0  # fp8 scale for normalized q
KS = 4.0   # fp8 scale for raw k
EXPA = 12102203.1616  # 2^23 / ln 2
EXPB = 1064866816.0   # 127*2^23 - 486408 (Schraudolph, log-mean-centered)

_CACHE = {}


def build_module(reps=1, use_f32r=True, phases=3, loop_n=0, no_pv=False,
                 dve_frac=0, pv_lag=1):
    """Build + compile the SPMD Bass module (one program, 8 cores)."""
    from contextlib import ExitStack

    import concourse.bacc as bacc
    import concourse.mybir as mybir
    import concourse.tile as tile
    from concourse.bass import ts
    from concourse.masks import make_identity

    dt = mybir.dt
    f32 = dt.float32
    bf16 = dt.bfloat16
    f8 = dt.float8e4
    AF = mybir.ActivationFunctionType
    ALU = mybir.AluOpType
    i32 = dt.int32
    DR = mybir.MatmulPerfMode.DoubleRow

    fr = dt.float32r if use_f32r else f32

    nc = bacc.Bacc("TRN2", target_bir_lowering=False, debug=False,
                   num_devices=NCORES)

    latT = nc.dram_tensor("latT", [D, L], f32, kind="ExternalInput")
    wqT_d = nc.dram_tensor("wqT", [D, D], f32, kind="ExternalInput")
    wkT_d = nc.dram_tensor("wkT", [D, D], f32, kind="ExternalInput")
    caug_d = nc.dram_tensor("caug", [128, 5 * (L // 128)], bf16,
                            kind="ExternalInput")
    pv_d = nc.dram_tensor("pv", [5, HALF], f32, kind="ExternalOutput")

    NLT = L // 128        # 32 m-tiles
    NQT = HALF // 128     # 16 q l-tiles
    NMB = L // 512        # 8 m-blocks
    NLB = HALF // 512     # 4 l-blocks

    with tile.TileContext(nc) as tc:
        loop = tc.For_i(0, loop_n, 1) if loop_n else None
        if loop is not None:
            loop.__enter__()
        for _rep in range(reps):
            with ExitStack() as ctx:
                persist = ctx.enter_context(tc.tile_pool(name="persist", bufs=1))

                # ---- load inputs (small weights first, lat chunks
                # interleaved across the two d-tiles so the first matmuls
                # can start after ~1MB) ----
                wq = [persist.tile([128, D], fr, tag=f"wq{i}", name=f"wq{i}") for i in range(2)]
                wk = [persist.tile([128, D], fr, tag=f"wk{i}", name=f"wk{i}") for i in range(2)]
                for i in range(2):
                    nc.sync.dma_start(out=wq[i], in_=wqT_d[i * 128:(i + 1) * 128, :].bitcast(fr))
                    nc.sync.dma_start(out=wk[i], in_=wkT_d[i * 128:(i + 1) * 128, :].bitcast(fr))
                caug = persist.tile([128, 5 * NLT], bf16, tag="caug")
                nc.sync.dma_start(out=caug, in_=caug_d[:, :])
                ident = persist.tile([128, 128], bf16, tag="ident")
                make_identity(nc, ident)
                ones = persist.tile([128, 1], f32, tag="ones")
                nc.vector.memset(ones, 1.0)

                lat = [persist.tile([128, L], fr, tag=f"lat{i}", name=f"lat{i}") for i in range(2)]
                chunks = [(0, 512), (512, 512), (1024, 1024), (2048, 1024),
                          (3072, 1024)]
                for off, size in chunks:
                    for i in range(2):
                        nc.sync.dma_start(
                            out=lat[i][:, off:off + size],
                            in_=latT[i * 128:(i + 1) * 128,
                                     off:off + size].bitcast(fr))

                # fp8 DoubleRow layouts: [K=128, k-subtile, len]
                qT = persist.tile([128, 2, HALF], f8, tag="qT", name="qT")
                kT = persist.tile([128, 2, L], f8, tag="kT", name="kT")
                q_all = persist.tile([128, NQT * D], f32, tag="q_all")
                ssq_q = persist.tile([128, NQT], f32, tag="ssq_q")
                inv_q = persist.tile([128, NQT], f32, tag="inv_q")
                inv_kT = persist.tile([128, NLT], f32, tag="inv_kT")
                inv_kA = persist.tile([128, NLT], f32, tag="inv_kA")

                with ExitStack() as p1:
                    big_ps = p1.enter_context(
                        tc.tile_pool(name="big_ps", bufs=3, space="PSUM"))
                    tp_ps = p1.enter_context(
                        tc.tile_pool(name="tp_ps", bufs=2, space="PSUM"))
                    kssq_ps = p1.enter_context(
                        tc.tile_pool(name="kssq_ps", bufs=1, space="PSUM"))
                    sm = p1.enter_context(tc.tile_pool(name="p1_small", bufs=4))
                    qh_pool = p1.enter_context(tc.tile_pool(name="qhat", bufs=3))
                    sq_pool = p1.enter_context(tc.tile_pool(name="k_sq", bufs=4))

                    # ---- phase 1q-A: raw q [l, e] + row sum-squares
                    # (ACT Square + accum_out straight from PSUM) ----
                    for lt in range(NQT):
                        qle = big_ps.tile([128, D], f32, tag="big", name="qle")
                        nc.tensor.matmul(qle, lat[0][:, ts(lt, 128)],
                                         wq[0], start=True, stop=False)
                        nc.tensor.matmul(qle, lat[1][:, ts(lt, 128)],
                                         wq[1], start=False, stop=True)
                        nc.vector.tensor_copy(out=q_all[:, ts(lt, D)], in_=qle)
                        junk = sm.tile([128, D], f32, tag="sqj")
                        nc.scalar.activation(junk, qle, AF.Square,
                                             accum_out=ssq_q[:, lt:lt + 1])
                    # inv_q = QS/||q_l||
                    nrm_q = persist.tile([128, NQT], f32, tag="nrm_q")
                    nc.scalar.activation(nrm_q, ssq_q, AF.Sqrt,
                                         scale=1.0 / (QS * QS))
                    nc.vector.reciprocal(inv_q, nrm_q)

                    # ---- phase 1q-B: normalize+scale to bf16, transpose to
                    # [e, l], store fp8 (copies alternate DVE/ACT) ----
                    for lt in range(NQT):
                        qhat = qh_pool.tile([128, D], bf16, tag="qhat")
                        nc.vector.tensor_scalar_mul(qhat, q_all[:, ts(lt, D)],
                                                    inv_q[:, lt:lt + 1])
                        for et in range(2):
                            tp = tp_ps.tile([128, 128], bf16, tag="tp")
                            nc.tensor.transpose(tp, qhat[:, ts(et, 128)], ident)
                            dst = qT[:, et, ts(lt, 128)]
                            if (2 * lt + et) % 2 == 0:
                                nc.vector.tensor_copy(out=dst, in_=tp)
                            else:
                                nc.scalar.copy(out=dst, in_=tp)

                    # ---- phase 1k: kT_raw [e, m] fp8 (scaled by KS); ssq
                    # via N=1 matmuls straight into the [m-tile] layout ----
                    kssq = kssq_ps.tile([128, NLT], f32, tag="kssq")
                    for mb in range(NMB):
                        sqs = []
                        for et in range(2):
                            kp = big_ps.tile([128, 512], f32, tag="big", name="kp")
                            nc.tensor.matmul(kp, wk[0][:, ts(et, 128)],
                                             lat[0][:, ts(mb, 512)],
                                             start=True, stop=False)
                            nc.tensor.matmul(kp, wk[1][:, ts(et, 128)],
                                             lat[1][:, ts(mb, 512)],
                                             start=False, stop=True)
                            # fp8 store (scaled): ACT Copy w/ scale
                            kslc = kT[:, et, ts(mb, 512)]
                            nc.scalar.mul(kslc, kp, KS)
                            # ssq of the QUANTIZED k (exactly what the score
                            # matmul consumes); sq = (KS*k_q)^2
                            sq = sq_pool.tile([128, 512], f32, tag="ksq")
                            nc.vector.tensor_mul(sq, kslc, kslc)
                            sqs.append(sq)
                        for j in range(4):
                            col = 4 * mb + j
                            for et in range(2):
                                nc.tensor.matmul(kssq[:, col:col + 1],
                                                 sqs[et][:, ts(j, 128)], ones,
                                                 start=(et == 0), stop=(et == 1))
                    # kssq = ||KS*k_q||^2; exp scale = INV_TEMP/(QS*||KS*k_q||)
                    nrm_k = persist.tile([128, NLT], f32, tag="nrm_k")
                    nc.scalar.activation(nrm_k, kssq, AF.Sqrt,
                                         scale=(QS / INV_TEMP) ** 2)
                    nc.vector.reciprocal(inv_kT, nrm_k)
                    if dve_frac:
                        nc.scalar.mul(inv_kA, inv_kT, EXPA)

                if phases < 3:
                    with tc.tile_pool(name="dbg", bufs=1) as dbg:
                        dtile = dbg.tile([5, HALF], f32, name="dtile")
                        nc.vector.tensor_copy(out=dtile,
                                              in_=kT[0:5, 0, 0:HALF])
                        nc.sync.dma_start(out=pv_d[:, :], in_=dtile)
                    continue

                # ---- phase 2: scores^T -> exp -> [coords|1]^T @ P^T ----
                # one DoubleRow fp8 matmul per (t, 512-l-block): contracts
                # all 256 e in a single pass. pv matmuls of tile t-1 are
                # emitted after the scores matmuls of tile t so PE never
                # waits on ACT's exp.
                with ExitStack() as p2:
                    sp_ps = p2.enter_context(
                        tc.tile_pool(name="sp_ps", bufs=3, space="PSUM"))
                    pv_ps = p2.enter_context(
                        tc.tile_pool(name="pv_ps", bufs=1, space="PSUM"))
                    p_pool = p2.enter_context(
                        tc.tile_pool(name="p_sb", bufs=4 + 2 * pv_lag))
                    ti_pool = p2.enter_context(tc.tile_pool(name="ti", bufs=3))
                    pv_all = pv_ps.tile([128, 512], f32, tag="pv")

                    def emit_pv(t, ptiles):
                        for lb in range(NLB):
                            prhs = ptiles[lb // 2][:, ts(lb % 2, 512)]
                            nc.tensor.matmul(
                                pv_all[32 * lb:32 * lb + 5, :],
                                caug[:, ts(t, 5)], prhs,
                                start=(t == 0), stop=(t == NLT - 1),
                                tile_position=(0, 32 * lb),
                                skip_group_check=True)

                    pending = []
                    for t in range(NLT):
                        cur = []
                        for j in range(2):
                            sp = sp_ps.tile([128, 1024], f32, tag="sp")
                            for h in range(2):
                                lb = 2 * j + h
                                nc.tensor.matmul(sp[:, ts(h, 512)],
                                                 kT[:, :, ts(t, 128)],
                                                 qT[:, :, ts(lb, 512)],
                                                 start=True, stop=True,
                                                 perf_mode=DR)
                            p = p_pool.tile([128, 1024], bf16, tag="p")
                            if j == 1 and (t % 4) < dve_frac:
                                # Schraudolph fast-exp on DVE: int32(A*s+B')
                                # bit pattern IS ~exp(s*scale) in f32
                                ti = ti_pool.tile([128, 1024], i32, tag="ti")
                                nc.vector.tensor_scalar(
                                    ti, sp, inv_kA[:, t:t + 1], EXPB,
                                    ALU.mult, ALU.add)
                                nc.vector.tensor_copy(out=p,
                                                      in_=ti.bitcast(f32))
                            else:
                                nc.scalar.activation(p, sp, AF.Exp,
                                                     scale=inv_kT[:, t:t + 1])
                            cur.append(p)
                        pending.append((t, cur))
                        if len(pending) > pv_lag and not no_pv:
                            te, pe = pending.pop(0)
                            emit_pv(te, pe)
                    out_sb = p2.enter_context(tc.tile_pool(name="out_sb", bufs=2))
                    if no_pv:
                        ot = out_sb.tile([5, 1024], f32, tag="otd", name="otd")
                        nc.vector.tensor_copy(out=ot, in_=pending[-1][1][1][0:5, :])
                        nc.sync.dma_start(out=pv_d[:, 0:1024], in_=ot)
                    else:
                        for te, pe in pending:
                            emit_pv(te, pe)
                        for lb in range(NLB):
                            ot = out_sb.tile([5, 512], f32, tag="ot")
                            nc.vector.tensor_copy(out=ot,
                                                  in_=pv_all[32 * lb:32 * lb + 5, :])
                            nc.sync.dma_start(out=pv_d[:, ts(lb, 512)], in_=ot)

        if loop is not None:
            loop.__exit__(None, None, None)
    nc.compile()
    return nc


def _get_module():
    if "nc" not in _CACHE:
        _CACHE["nc"] = build_module()
    return _CACHE["nc"]


def make_in_maps(latents, current_coords, Wq, Wk):
    """Per-core input dicts. Core c -> batch c//2, query half c%2 (rolled
    so own query rows are always columns 0:2048)."""
    import ml_dtypes
    bf16 = ml_dtypes.bfloat16
    latents = np.asarray(latents, np.float32)
    coords = np.asarray(current_coords, np.float32)
    wqT = np.ascontiguousarray(np.asarray(Wq, np.float32).T)
    wkT = np.ascontiguousarray(np.asarray(Wk, np.float32).T)
    NLT = L // 128
    in_maps = []
    for c in range(NCORES):
        b, h = divmod(c, 2)
        lat_b = np.roll(latents[b], -HALF * h, axis=0)
        coo_b = np.roll(coords[b], -HALF * h, axis=0)
        aug = np.concatenate([coo_b, np.ones((L, 1), np.float32)], axis=1)
        # [128, NLT, 3]: partition = within-tile index
        a = aug.reshape(NLT, 128, 3).transpose(1, 0, 2)
        hi = a.astype(bf16)
        lo = (a - hi.astype(np.float32)).astype(bf16)
        # merged stationary per tile: x_hi, y_hi, 1, x_lo, y_lo
        caug = np.concatenate([hi, lo[..., :2]], axis=-1).reshape(128, -1)
        in_maps.append({
            "latT": np.ascontiguousarray(lat_b.T),
            "wqT": wqT,
            "wkT": wkT,
            "caug": np.ascontiguousarray(caug),
        })
    return in_maps


def postprocess(results, current_coords, alpha):
    """Assemble (new_coords, displacement) from per-core
    pv = [num_x_hi; num_y_hi; den; num_x_lo; num_y_lo]."""
    coords = np.asarray(current_coords, np.float32)
    new_coords = np.empty((B, L, 2), np.float32)
    for c in range(NCORES):
        b, h = divmod(c, 2)
        pv = results[c]["pv"]
        num = pv[0:2, :] + pv[3:5, :]
        wc = (num / pv[2:3, :]).T  # [2048, 2] = (W @ coords) rows
        rows = slice(h * HALF, (h + 1) * HALF)
        new_coords[b, rows] = alpha * wc + (1.0 - alpha) * coords[b, rows]
    displacement = new_coords - coords
    return new_coords, displacement


def kernel(latents, current_coords, Wq, Wk, alpha_raw, layer_idx=None):
    from concourse.bass_utils import run_bass_kernel_spmd

    nc = _get_module()
    in_maps = make_in_maps(latents, current_coords, Wq, Wk)
    res = run_bass_kernel_spmd(nc, in_maps, list(range(NCORES)))
    alpha = np.float32(1.0 / (1.0 + np.exp(-np.float64(np.asarray(alpha_raw)))))
    return postprocess(res.results, current_coords, alpha)
